# revision 9
# baseline (speedup 1.0000x reference)
"""Causal self-attention with RoPE on 8 Trainium2 NeuronCores.

Sharding: tensor-parallel over heads (2 heads/core) for QKV+attention,
then an AllToAll redistributes y^T from head-shards to token-shards and
each core projects its 512-token slice with the full W_proj.

v1 pipeline rewrite:
 - software-pipelined emission over units (rep, batch): stage u emits
   qkv+attention of unit u interleaved with the projection of unit u-2,
   with unit u's AllToAll kicked off at stage end so the collective
   overlaps the whole next stage.
 - attention operands (q^T, k^T, v, exp weights) and the a2a payload /
   projection run in bf16; QKV projection stays fp32r.
 - W_proj lives in SBUF as a constant (no per-rep reload).
 - x transposes run in fp32r (1.5 cyc/row vs 2.0 for fp32).
"""

import math

import numpy as np

import concourse.bass as bass
import concourse.mybir as mybir
import concourse.tile as tile
from concourse import bacc
from concourse.bass_utils import run_bass_kernel_spmd

# Problem shape (hardcoded per contest rules).
B, T, D = 2, 2048, 1024
H, DH = 16, 64
ROPE_BASE = 10000.0
N_CORES = 8
P = 128
N_STRIPS = T // 512                    # 4 strips of 512 tokens per batch
TOK = B * T                            # 4096 flat tokens
TOK_PER_CORE = TOK // N_CORES          # 512
DC = D // P                            # 8 contraction chunks

FP32 = mybir.dt.float32
FP32R = mybir.dt.float32r
BF16 = mybir.dt.bfloat16
AF = mybir.ActivationFunctionType
ALU = mybir.AluOpType


class _Ctx:
    """Bag of handles shared by the stage emitters."""

    def __init__(self, nc, d, consts, pools):
        self.nc = nc
        self.d = d
        self.c = consts
        self.p = pools


def _emit_qkv_strip(ctx, u, s):
    """Load x strip, transpose, QKV matmuls, RoPE -> qt/kt, V -> va/vb."""
    nc, d, c, p = ctx.nc, ctx.d, ctx.c, ctx.p
    b = u % B
    qt, kt = c[f"qt{u % 2}"], c[f"kt{u % 2}"]
    va, vb = c[f"va{u % 2}"], c[f"vb{u % 2}"]

    xns = []
    for tt in range(4):
        xn = p["spool"].tile([P, D], FP32, tag=f"xn{tt}", name="xn")
        r0 = b * T + s * 512 + tt * P
        nc.sync.dma_start(xn[:, 0:512], d["x"][r0 : r0 + P, 0:512])
        nc.sync.dma_start(xn[:, 512:D], d["x"][r0 : r0 + P, 512:D])
        xns.append(xn)

    xtc = p["spool"].tile([P, DC, 512], FP32R, tag="xtc")
    for dc in range(DC):
        ptile = p["ptr"].tile([P, 4, P], FP32, tag="ptr")
        for tt in range(4):
            nc.tensor.transpose(
                ptile[:, tt], xns[tt][:, dc * P : (dc + 1) * P], c["ident"][:]
            )
        nc.vector.tensor_copy(xtc[:, dc], ptile[:])

    sl = slice(s * 512, (s + 1) * 512)
    # Q and K: accumulate in one PSUM bank each, then RoPE into qt/kt.
    for i, dst in ((0, qt), (1, kt)):
        pm = p["pm"].tile([P, 512], FP32, tag="pm")
        for dc in range(DC):
            nc.tensor.matmul(
                pm[:],
                c["w_sb"]["qkv"[i]][:, dc],
                xtc[:, dc],
                start=(dc == 0),
                stop=(dc == DC - 1),
            )
        raw = p["spool"].tile([P, 512], FP32, tag=f"raw{i}")
        nc.vector.tensor_copy(raw[:], pm[:])
        perm = p["spool"].tile([P, 512], FP32, tag=f"perm{i}")
        for blk in range(4):
            p0 = blk * 32
            src = p0 + 32 if blk % 2 == 0 else p0 - 32
            nc.sync.dma_start(perm[p0 : p0 + 32, :], raw[src : src + 32, :])
        nc.gpsimd.tensor_tensor(raw[:], raw[:], c["cos_sb"][:, sl], ALU.mult)
        nc.gpsimd.tensor_tensor(perm[:], perm[:], c["sin_sb"][:, sl], ALU.mult)
        nc.vector.tensor_tensor(dst[:, sl], raw[:], perm[:], ALU.add)

    # V: accumulate, transpose to token-major, split per head (bf16).
    pm = p["pm"].tile([P, 512], FP32, tag="pm")
    for dc in range(DC):
        nc.tensor.matmul(
            pm[:],
            c["w_sb"]["v"][:, dc],
            xtc[:, dc],
            start=(dc == 0),
            stop=(dc == DC - 1),
        )
    vt = p["spool"].tile([P, 512], FP32, tag="vt")
    nc.vector.tensor_copy(vt[:], pm[:])
    ptile2 = p["ptr"].tile([P, 4, P], FP32, tag="ptr")
    for tt in range(4):
        nc.tensor.transpose(
            ptile2[:, tt], vt[:, tt * P : (tt + 1) * P], c["ident"][:]
        )
    nc.vector.tensor_copy(va[:, s * 4 : s * 4 + 4, 0:64], ptile2[:, :, 0:64])
    nc.vector.tensor_copy(vb[:, s * 4 : s * 4 + 4, 0:64], ptile2[:, :, 64:128])


def _emit_attn_block(ctx, u, s):
    """Attention for query strip s, both heads interleaved per key chunk."""
    nc, d, c, p = ctx.nc, ctx.d, ctx.c, ctx.p
    qt, kt = c[f"qt{u % 2}"], c[f"kt{u % 2}"]
    v_h = (c[f"va{u % 2}"], c[f"vb{u % 2}"])
    y2t = (c[f"y2t0_{u % 2}"], c[f"y2t1_{u % 2}"])
    qsl = slice(s * 512, (s + 1) * 512)
    jmax = 4 * s + 3
    pyts = [p["py"].tile([65, 512], FP32, tag="py", name="pyt") for _ in range(2)]
    for j in range(jmax + 1):
        col0 = max(0, P * (j - 4 * s))
        w = 512 - col0
        pts = []
        for h in range(2):
            ph = 64 * h
            pss = p["ps"].tile([P, 512], FP32, tag="ps")
            nc.tensor.matmul(
                pss[:, 0:w],
                kt[ph : ph + 64, j * P : (j + 1) * P],
                qt[ph : ph + 64, s * 512 + col0 : (s + 1) * 512],
                start=True,
                stop=True,
            )
            pt = p["ptp"].tile([P, 512], BF16, tag="pt")
            nc.scalar.activation(
                pt[:, 0:w], pss[:, 0:w], AF.Exp, scale=1.0 / math.sqrt(DH)
            )
            if j >= 4 * s:
                nc.gpsimd.affine_select(
                    out=pt[:, 0:P],
                    in_=pt[:, 0:P],
                    compare_op=ALU.is_ge,
                    fill=0.0,
                    base=0,
                    channel_multiplier=-1,
                    pattern=[[1, P]],
                )
            pts.append(pt)
        for h in range(2):
            nc.tensor.matmul(
                pyts[h][:, col0:512],
                v_h[h][:, j, :],
                pts[h][:, 0:w],
                start=(j == 0),
                stop=(j == jmax),
            )
    for h in range(2):
        r65 = p["apool"].tile([1, 512], FP32, tag="r65")
        nc.vector.reciprocal(r65[:], pyts[h][64:65, :])
        r_dram = p["dram"].tile([1, 512], FP32, tag="r_dram", name="r_dram")
        nc.sync.dma_start(r_dram[:], r65[:])
        rb = p["apool"].tile([64, 512], FP32, tag="rb")
        nc.sync.dma_start(rb[:], r_dram[:].to_broadcast((64, 512)))
        nc.vector.tensor_tensor(y2t[h][:, qsl], pyts[h][0:64, :], rb[:], ALU.mult)


def _emit_a2a(ctx, u):
    """Head-shards -> 256-token shards for unit u (bf16 payload)."""
    nc, d, c, p = ctx.nc, ctx.d, ctx.c, ctx.p
    y2t = (c[f"y2t0_{u % 2}"], c[f"y2t1_{u % 2}"])
    a2a_in = p["dram"].tile([N_CORES * P, 256], BF16, tag="a2a_in", name="a2a_in")
    a2a_out = p["dram"].tile([N_CORES * P, 256], BF16, tag="a2a_out", name="a2a_out")
    for j in range(N_CORES):
        jsl = slice(j * 256, (j + 1) * 256)
        nc.sync.dma_start(a2a_in[j * P : j * P + 64, :], y2t[0][:, jsl])
        nc.sync.dma_start(a2a_in[j * P + 64 : (j + 1) * P, :], y2t[1][:, jsl])
    nc.gpsimd.collective_compute(
        "AllToAll",
        ALU.bypass,
        replica_groups=[list(range(N_CORES))],
        ins=[a2a_in.opt()],
        outs=[a2a_out.opt()],
    )
    c[f"a2a_out{u % 2}"] = a2a_out


def _emit_proj_chunk(ctx, u, s):
    """1/4 of the projection of unit u: 512 output columns (s selects them)."""
    nc, d, c, p = ctx.nc, ctx.d, ctx.c, ctx.p
    b = u % B
    if s == 0:
        yt_sb = p["opool"].tile([P, DC, 256], BF16, tag="yt")
        nc.sync.dma_start(
            yt_sb[:],
            c[f"a2a_out{u % 2}"][:].rearrange("(o p) t -> p o t", p=P),
        )
        c[f"yt{u % 2}"] = yt_sb
    yt_sb = c[f"yt{u % 2}"]
    tt, c0 = s // 2, (s % 2) * 512
    pmo = p["pm"].tile([P, 512], FP32, tag="pm")
    for dc in range(DC):
        nc.tensor.matmul(
            pmo[:],
            yt_sb[:, dc, tt * P : (tt + 1) * P],
            c["wp_sb"][:, dc, c0 : c0 + 512],
            start=(dc == 0),
            stop=(dc == DC - 1),
        )
    ob = p["opool"].tile([P, 512], FP32, tag="ob")
    nc.vector.tensor_copy(ob[:], pmo[:])
    nc.sync.dma_start(
        d["out"][b * 256 + tt * P : b * 256 + (tt + 1) * P, c0 : c0 + 512],
        ob[:],
    )


def _build_program(reps=1):
    nc = bacc.Bacc(None, target_bir_lowering=False, debug=False)

    d = {
        "x": nc.dram_tensor("x", [TOK, D], FP32, kind="ExternalInput"),
        "wq": nc.dram_tensor("wq", [D, P], FP32, kind="ExternalInput"),
        "wk": nc.dram_tensor("wk", [D, P], FP32, kind="ExternalInput"),
        "wv": nc.dram_tensor("wv", [D, P], FP32, kind="ExternalInput"),
        "wp": nc.dram_tensor("wp", [D, D], FP32, kind="ExternalInput"),
        "cos": nc.dram_tensor("cos", [P, T], FP32, kind="ExternalInput"),
        "sin": nc.dram_tensor("sin", [P, T], FP32, kind="ExternalInput"),
        "ident": nc.dram_tensor("ident", [P, P], FP32, kind="ExternalInput"),
        "out": nc.dram_tensor("out", [TOK_PER_CORE, D], FP32, kind="ExternalOutput"),
    }

    with tile.TileContext(nc) as tc:
        with (
            tc.tile_pool(name="const", bufs=1) as cpool,
            tc.tile_pool(name="unit", bufs=1) as upool,
            tc.tile_pool(name="spool", bufs=2) as spool,
            tc.tile_pool(name="ptp", bufs=4) as ptp,
            tc.tile_pool(name="apool", bufs=2) as apool,
            tc.tile_pool(name="opool", bufs=2) as opool,
            tc.tile_pool(name="ptr", bufs=2, space="PSUM") as ptr,
            tc.tile_pool(name="pm", bufs=2, space="PSUM") as pm,
            tc.tile_pool(name="ps", bufs=2, space="PSUM") as ps,
            tc.tile_pool(name="py", bufs=2, space="PSUM") as py,
            tc.tile_pool(name="dram", bufs=2, space="DRAM") as dram,
        ):
            ident = cpool.tile([P, P], FP32)
            nc.sync.dma_start(ident[:], d["ident"][:])

            w_sb = {}
            for name in ("q", "k", "v"):
                w_sb[name] = cpool.tile(
                    [P, DC, P], FP32R, tag=f"w{name}", name=f"w{name}"
                )
                nc.sync.dma_start(
                    w_sb[name][:],
                    d[f"w{name}"][:].rearrange("(o p) j -> p o j", p=P).bitcast(FP32R),
                )
            # W_proj resident in SBUF as bf16: [128, DC, 1024]. Stage the
            # fp32->bf16 conversion through a spool tile (reused later by
            # the QKV strips) so no permanent fp32 copy lives in SBUF.
            wp_sb = cpool.tile([P, DC, D], BF16, tag="wp_sb")
            for half in range(2):
                csl = slice(half * 512, (half + 1) * 512)
                wstage = spool.tile([P, DC, 512], FP32R, tag="xtc")
                nc.sync.dma_start(
                    wstage[:],
                    d["wp"][:, csl].rearrange("(o p) j -> p o j", p=P).bitcast(FP32R),
                )
                nc.gpsimd.tensor_copy(wp_sb[:, :, csl], wstage[:].bitcast(FP32))

            cos_sb = cpool.tile([P, T], FP32)
            sin_sb = cpool.tile([P, T], FP32)
            nc.sync.dma_start(cos_sb[:], d["cos"][:])
            nc.sync.dma_start(sin_sb[:], d["sin"][:])

            consts = dict(
                ident=ident, w_sb=w_sb, wp_sb=wp_sb,
                cos_sb=cos_sb, sin_sb=sin_sb,
            )
            # Per-unit double-buffered tiles (parity-indexed).
            for par in range(2):
                consts[f"qt{par}"] = upool.tile([P, T], BF16, tag=f"qt{par}", name="qt")
                consts[f"kt{par}"] = upool.tile([P, T], BF16, tag=f"kt{par}", name="kt")
                for vn in ("va", "vb"):
                    v = upool.tile([P, T // P, 65], BF16, tag=f"{vn}{par}", name="v")
                    consts[f"{vn}{par}"] = v
                    nc.gpsimd.memset(v[:, :, 64], 1.0)
                consts[f"y2t0_{par}"] = upool.tile([64, T], BF16, tag=f"y2t0_{par}", name="y2t0")
                consts[f"y2t1_{par}"] = upool.tile([64, T], BF16, tag=f"y2t1_{par}", name="y2t1")

            pools = dict(
                spool=spool, ptp=ptp, apool=apool, opool=opool,
                ptr=ptr, pm=pm, ps=ps, py=py, dram=dram,
            )
            ctx = _Ctx(nc, d, consts, pools)

            U = reps * B
            for u in range(U):
                for s in range(N_STRIPS):
                    _emit_qkv_strip(ctx, u, s)
                    _emit_attn_block(ctx, u, s)
                    if u >= 2:
                        _emit_proj_chunk(ctx, u - 2, s)
                _emit_a2a(ctx, u)
            # Tail: projections of the last two units.
            for u in range(max(0, U - 2), U):
                for s in range(N_STRIPS):
                    _emit_proj_chunk(ctx, u, s)

    nc.compile()
    return nc


_NC_CACHE = {}


def _get_program(reps=1):
    if reps not in _NC_CACHE:
        _NC_CACHE[reps] = _build_program(reps)
    return _NC_CACHE[reps]


def _host_tables():
    inv_freq = 1.0 / (ROPE_BASE ** (np.arange(0, DH, 2, dtype=np.float32) / DH))
    t = np.arange(T, dtype=np.float32)
    freqs = np.outer(t, inv_freq).astype(np.float32)  # (T, 32)
    cos_t = np.cos(freqs).T                           # (32, T)
    sin_t = np.sin(freqs).T
    cos = np.empty((P, T), np.float32)
    sin = np.empty((P, T), np.float32)
    for blk in range(4):
        cos[blk * 32 : (blk + 1) * 32] = cos_t
        # rotate_half: row p<32 pairs with -q[p+32]; row p>=32 with +q[p-32]
        sgn = -1.0 if blk % 2 == 0 else 1.0
        sin[blk * 32 : (blk + 1) * 32] = sgn * sin_t
    return cos, sin


def make_in_maps(x, W_qkv, W_proj):
    x = np.asarray(x, np.float32).reshape(TOK, D)
    W_qkv = np.asarray(W_qkv, np.float32)
    W_proj = np.asarray(W_proj, np.float32)
    cos, sin = _host_tables()

    in_maps = []
    for c in range(N_CORES):
        j0 = c * P
        in_maps.append(
            {
                "x": x,
                "wq": np.ascontiguousarray(W_qkv[:, j0 : j0 + P]),
                "wk": np.ascontiguousarray(W_qkv[:, D + j0 : D + j0 + P]),
                "wv": np.ascontiguousarray(W_qkv[:, 2 * D + j0 : 2 * D + j0 + P]),
                "wp": W_proj,
                "cos": cos,
                "sin": sin,
                "ident": np.eye(P, dtype=np.float32),
            }
        )
    return in_maps


def kernel(x, W_qkv, W_proj):
    in_maps = make_in_maps(x, W_qkv, W_proj)
    nc = _get_program()
    res = run_bass_kernel_spmd(nc, in_maps, list(range(N_CORES)))
    return assemble([res.results[c]["out"] for c in range(N_CORES)])


def assemble(outs):
    full = np.empty((B, T, D), np.float32)
    for c in range(N_CORES):
        o = outs[c]
        for b in range(B):
            full[b, 256 * c : 256 * (c + 1)] = o[b * 256 : (b + 1) * 256]
    return full


# revision 11
# speedup vs baseline: 2.6540x; 2.6540x over previous
"""Causal self-attention with RoPE on 8 Trainium2 NeuronCores.

Sharding: tensor-parallel over heads (2 heads/core) for QKV+attention,
then an AllToAll redistributes y^T from head-shards to token-shards and
each core projects its 512-token slice with the full W_proj.

v1 pipeline rewrite:
 - software-pipelined emission over units (rep, batch): stage u emits
   qkv+attention of unit u interleaved with the projection of unit u-2,
   with unit u's AllToAll kicked off at stage end so the collective
   overlaps the whole next stage.
 - attention operands (q^T, k^T, v, exp weights) and the a2a payload /
   projection run in bf16; QKV projection stays fp32r.
 - W_proj lives in SBUF as a constant (no per-rep reload).
 - x transposes run in fp32r (1.5 cyc/row vs 2.0 for fp32).
"""

import math

import numpy as np

import concourse.bass as bass
import concourse.mybir as mybir
import concourse.tile as tile
from concourse import bacc
from concourse.bass_utils import run_bass_kernel_spmd

# Problem shape (hardcoded per contest rules).
B, T, D = 2, 2048, 1024
H, DH = 16, 64
ROPE_BASE = 10000.0
N_CORES = 8
P = 128
N_STRIPS = T // 512                    # 4 strips of 512 tokens per batch
TOK = B * T                            # 4096 flat tokens
TOK_PER_CORE = TOK // N_CORES          # 512
DC = D // P                            # 8 contraction chunks

FP32 = mybir.dt.float32
FP32R = mybir.dt.float32r
BF16 = mybir.dt.bfloat16
AF = mybir.ActivationFunctionType
ALU = mybir.AluOpType


class _Ctx:
    """Bag of handles shared by the stage emitters."""

    def __init__(self, nc, d, consts, pools):
        self.nc = nc
        self.d = d
        self.c = consts
        self.p = pools


def _emit_loads(ctx, u, s, slot):
    """Fire x-strip DMAs one strip ahead of use (SP ring, wait-free)."""
    nc, d, c, p = ctx.nc, ctx.d, ctx.c, ctx.p
    b = u % B
    xns = []
    for tt in range(4):
        xn = p["spool"].tile([P, D], FP32, tag=f"xn{tt}", name="xn")
        r0 = b * T + s * 512 + tt * P
        nc.sync.dma_start(xn[:, 0:512], d["x"][r0 : r0 + P, 0:512])
        nc.sync.dma_start(xn[:, 512:D], d["x"][r0 : r0 + P, 512:D])
        xns.append(xn)
    c[f"xns{slot}"] = xns


def _emit_qkv_strip(ctx, u, s):
    """Transpose, QKV matmuls, RoPE -> qt/kt, V -> va/vb for strip s."""
    nc, d, c, p = ctx.nc, ctx.d, ctx.c, ctx.p
    qt, kt = c[f"qt{u % 2}"], c[f"kt{u % 2}"]
    va, vb = c[f"va{u % 2}"], c[f"vb{u % 2}"]
    xns = c[f"xns{(u * N_STRIPS + s) % 2}"]

    xtc = p["spool"].tile([P, DC, 512], FP32R, tag="xtc")
    for dc in range(DC):
        ptile = p["ptr"].tile([P, 4, P], FP32, tag="ptr")
        for tt in range(4):
            nc.tensor.transpose(
                ptile[:, tt], xns[tt][:, dc * P : (dc + 1) * P], c["ident"][:]
            )
        nc.vector.tensor_copy(xtc[:, dc], ptile[:])

    sl = slice(s * 512, (s + 1) * 512)
    # Q and K: accumulate in one PSUM bank each, then RoPE into qt/kt.
    for i, dst in ((0, qt), (1, kt)):
        pm = p["pm"].tile([P, 512], FP32, tag="pm")
        for dc in range(DC):
            nc.tensor.matmul(
                pm[:],
                c["w_sb"]["qkv"[i]][:, dc],
                xtc[:, dc],
                start=(dc == 0),
                stop=(dc == DC - 1),
            )
        raw = p["spool"].tile([P, 512], FP32, tag=f"raw{i}")
        nc.vector.tensor_copy(raw[:], pm[:])
        perm = p["spool"].tile([P, 512], FP32, tag=f"perm{i}")
        for blk in range(4):
            p0 = blk * 32
            src = p0 + 32 if blk % 2 == 0 else p0 - 32
            nc.scalar.dma_start(perm[p0 : p0 + 32, :], raw[src : src + 32, :])
        nc.gpsimd.tensor_tensor(raw[:], raw[:], c["cos_sb"][:, sl], ALU.mult)
        nc.gpsimd.tensor_tensor(perm[:], perm[:], c["sin_sb"][:, sl], ALU.mult)
        nc.gpsimd.tensor_tensor(dst[:, sl], raw[:], perm[:], ALU.add)

    # V: accumulate, transpose to token-major, split per head (bf16).
    pm = p["pm"].tile([P, 512], FP32, tag="pm")
    for dc in range(DC):
        nc.tensor.matmul(
            pm[:],
            c["w_sb"]["v"][:, dc],
            xtc[:, dc],
            start=(dc == 0),
            stop=(dc == DC - 1),
        )
    vt = p["spool"].tile([P, 512], FP32, tag="vt")
    nc.vector.tensor_copy(vt[:], pm[:])
    ptile2 = p["ptr"].tile([P, 4, P], FP32, tag="ptr")
    for tt in range(4):
        nc.tensor.transpose(
            ptile2[:, tt], vt[:, tt * P : (tt + 1) * P], c["ident"][:]
        )
    nc.vector.tensor_copy(va[:, s * 4 : s * 4 + 4, 0:64], ptile2[:, :, 0:64])
    nc.vector.tensor_copy(vb[:, s * 4 : s * 4 + 4, 0:64], ptile2[:, :, 64:128])


def _emit_attn_block(ctx, u, s):
    """Attention for query strip s, both heads interleaved per key chunk."""
    nc, d, c, p = ctx.nc, ctx.d, ctx.c, ctx.p
    qt, kt = c[f"qt{u % 2}"], c[f"kt{u % 2}"]
    v_h = (c[f"va{u % 2}"], c[f"vb{u % 2}"])
    y2t = (c[f"y2t0_{u % 2}"], c[f"y2t1_{u % 2}"])
    qsl = slice(s * 512, (s + 1) * 512)
    jmax = 4 * s + 3
    pyts = [p["py"].tile([65, 512], FP32, tag="py", name="pyt") for _ in range(2)]
    for j in range(jmax + 1):
        col0 = max(0, P * (j - 4 * s))
        w = 512 - col0
        pts = []
        for h in range(2):
            ph = 64 * h
            pss = p["ps"].tile([P, 512], FP32, tag="ps")
            nc.tensor.matmul(
                pss[:, 0:w],
                kt[ph : ph + 64, j * P : (j + 1) * P],
                qt[ph : ph + 64, s * 512 + col0 : (s + 1) * 512],
                start=True,
                stop=True,
            )
            pt = p["ptp"].tile([P, 512], BF16, tag="pt")
            nc.scalar.activation(
                pt[:, 0:w], pss[:, 0:w], AF.Exp, scale=1.0 / math.sqrt(DH)
            )
            if j >= 4 * s:
                nc.gpsimd.affine_select(
                    out=pt[:, 0:P],
                    in_=pt[:, 0:P],
                    compare_op=ALU.is_ge,
                    fill=0.0,
                    base=0,
                    channel_multiplier=-1,
                    pattern=[[1, P]],
                )
            pts.append(pt)
        for h in range(2):
            nc.tensor.matmul(
                pyts[h][:, col0:512],
                v_h[h][:, j, :],
                pts[h][:, 0:w],
                start=(j == 0),
                stop=(j == jmax),
            )
    for h in range(2):
        r65 = p["apool"].tile([1, 512], FP32, tag="r65")
        nc.vector.reciprocal(r65[:], pyts[h][64:65, :])
        r_dram = p["dram"].tile([1, 512], FP32, tag="r_dram", name="r_dram")
        nc.scalar.dma_start(r_dram[:], r65[:])
        rb = p["apool"].tile([64, 512], FP32, tag="rb")
        nc.scalar.dma_start(rb[:], r_dram[:].to_broadcast((64, 512)))
        nc.vector.tensor_tensor(y2t[h][:, qsl], pyts[h][0:64, :], rb[:], ALU.mult)


def _emit_a2a(ctx, u):
    """Head-shards -> 256-token shards for unit u (bf16 payload)."""
    nc, d, c, p = ctx.nc, ctx.d, ctx.c, ctx.p
    y2t = (c[f"y2t0_{u % 2}"], c[f"y2t1_{u % 2}"])
    a2a_in = p["dram"].tile([N_CORES * P, 256], BF16, tag="a2a_in", name="a2a_in")
    a2a_out = p["dram"].tile([N_CORES * P, 256], BF16, tag="a2a_out", name="a2a_out")
    for j in range(N_CORES):
        jsl = slice(j * 256, (j + 1) * 256)
        nc.scalar.dma_start(a2a_in[j * P : j * P + 64, :], y2t[0][:, jsl])
        nc.scalar.dma_start(a2a_in[j * P + 64 : (j + 1) * P, :], y2t[1][:, jsl])
    nc.gpsimd.collective_compute(
        "AllToAll",
        ALU.bypass,
        replica_groups=[list(range(N_CORES))],
        ins=[a2a_in.opt()],
        outs=[a2a_out.opt()],
    )
    c[f"a2a_out{u % 2}"] = a2a_out


def _emit_yt_prefetch(ctx, u):
    """Load unit u's a2a output for projection (SP ring, collective done)."""
    nc, d, c, p = ctx.nc, ctx.d, ctx.c, ctx.p
    yt_sb = p["opool"].tile([P, DC, 256], BF16, tag="yt")
    nc.sync.dma_start(
        yt_sb[:],
        c[f"a2a_out{u % 2}"][:].rearrange("(o p) t -> p o t", p=P),
    )
    c[f"yt{u % 2}"] = yt_sb


def _emit_proj_chunk(ctx, u, s):
    """1/4 of the projection of unit u: 512 output columns (s selects them)."""
    nc, d, c, p = ctx.nc, ctx.d, ctx.c, ctx.p
    b = u % B
    yt_sb = c[f"yt{u % 2}"]
    tt, c0 = s // 2, (s % 2) * 512
    pmo = p["pm"].tile([P, 512], FP32, tag="pm")
    for dc in range(DC):
        nc.tensor.matmul(
            pmo[:],
            yt_sb[:, dc, tt * P : (tt + 1) * P],
            c["wp_sb"][:, dc, c0 : c0 + 512],
            start=(dc == 0),
            stop=(dc == DC - 1),
        )
    ob = p["opool"].tile([P, 512], FP32, tag="ob")
    nc.vector.tensor_copy(ob[:], pmo[:])
    nc.scalar.dma_start(
        d["out"][b * 256 + tt * P : b * 256 + (tt + 1) * P, c0 : c0 + 512],
        ob[:],
    )


def _build_program(reps=1):
    nc = bacc.Bacc(None, target_bir_lowering=False, debug=False)

    d = {
        "x": nc.dram_tensor("x", [TOK, D], FP32, kind="ExternalInput"),
        "wq": nc.dram_tensor("wq", [D, P], FP32, kind="ExternalInput"),
        "wk": nc.dram_tensor("wk", [D, P], FP32, kind="ExternalInput"),
        "wv": nc.dram_tensor("wv", [D, P], FP32, kind="ExternalInput"),
        "wp": nc.dram_tensor("wp", [D, D], FP32, kind="ExternalInput"),
        "cos": nc.dram_tensor("cos", [P, T], FP32, kind="ExternalInput"),
        "sin": nc.dram_tensor("sin", [P, T], FP32, kind="ExternalInput"),
        "ident": nc.dram_tensor("ident", [P, P], FP32, kind="ExternalInput"),
        "out": nc.dram_tensor("out", [TOK_PER_CORE, D], FP32, kind="ExternalOutput"),
    }

    with tile.TileContext(nc) as tc:
        with (
            tc.tile_pool(name="const", bufs=1) as cpool,
            tc.tile_pool(name="unit", bufs=1) as upool,
            tc.tile_pool(name="spool", bufs=2) as spool,
            tc.tile_pool(name="ptp", bufs=4) as ptp,
            tc.tile_pool(name="apool", bufs=2) as apool,
            tc.tile_pool(name="opool", bufs=2) as opool,
            tc.tile_pool(name="ptr", bufs=2, space="PSUM") as ptr,
            tc.tile_pool(name="pm", bufs=2, space="PSUM") as pm,
            tc.tile_pool(name="ps", bufs=2, space="PSUM") as ps,
            tc.tile_pool(name="py", bufs=2, space="PSUM") as py,
            tc.tile_pool(name="dram", bufs=2, space="DRAM") as dram,
        ):
            ident = cpool.tile([P, P], FP32)
            nc.sync.dma_start(ident[:], d["ident"][:])

            w_sb = {}
            for name in ("q", "k", "v"):
                w_sb[name] = cpool.tile(
                    [P, DC, P], FP32R, tag=f"w{name}", name=f"w{name}"
                )
                nc.sync.dma_start(
                    w_sb[name][:],
                    d[f"w{name}"][:].rearrange("(o p) j -> p o j", p=P).bitcast(FP32R),
                )
            # W_proj resident in SBUF as bf16: [128, DC, 1024]. Stage the
            # fp32->bf16 conversion through a spool tile (reused later by
            # the QKV strips) so no permanent fp32 copy lives in SBUF.
            wp_sb = cpool.tile([P, DC, D], BF16, tag="wp_sb")
            for half in range(2):
                csl = slice(half * 512, (half + 1) * 512)
                wstage = spool.tile([P, DC, 512], FP32R, tag="xtc")
                nc.sync.dma_start(
                    wstage[:],
                    d["wp"][:, csl].rearrange("(o p) j -> p o j", p=P).bitcast(FP32R),
                )
                nc.gpsimd.tensor_copy(wp_sb[:, :, csl], wstage[:].bitcast(FP32))

            cos_sb = cpool.tile([P, T], FP32)
            sin_sb = cpool.tile([P, T], FP32)
            nc.sync.dma_start(cos_sb[:], d["cos"][:])
            nc.sync.dma_start(sin_sb[:], d["sin"][:])

            consts = dict(
                ident=ident, w_sb=w_sb, wp_sb=wp_sb,
                cos_sb=cos_sb, sin_sb=sin_sb,
            )
            # Per-unit double-buffered tiles (parity-indexed).
            for par in range(2):
                consts[f"qt{par}"] = upool.tile([P, T], BF16, tag=f"qt{par}", name="qt")
                consts[f"kt{par}"] = upool.tile([P, T], BF16, tag=f"kt{par}", name="kt")
                for vn in ("va", "vb"):
                    v = upool.tile([P, T // P, 65], BF16, tag=f"{vn}{par}", name="v")
                    consts[f"{vn}{par}"] = v
                    nc.gpsimd.memset(v[:, :, 64], 1.0)
                consts[f"y2t0_{par}"] = upool.tile([64, T], BF16, tag=f"y2t0_{par}", name="y2t0")
                consts[f"y2t1_{par}"] = upool.tile([64, T], BF16, tag=f"y2t1_{par}", name="y2t1")

            pools = dict(
                spool=spool, ptp=ptp, apool=apool, opool=opool,
                ptr=ptr, pm=pm, ps=ps, py=py, dram=dram,
            )
            ctx = _Ctx(nc, d, consts, pools)

            U = reps * B
            strips = [(u, s) for u in range(U) for s in range(N_STRIPS)]
            _emit_loads(ctx, *strips[0], 0)
            for u in range(U):
                if u >= 2:
                    _emit_yt_prefetch(ctx, u - 2)
                for s in range(N_STRIPS):
                    g = u * N_STRIPS + s
                    if g + 1 < len(strips):
                        _emit_loads(ctx, *strips[g + 1], (g + 1) % 2)
                    _emit_qkv_strip(ctx, u, s)
                    _emit_attn_block(ctx, u, s)
                    if u >= 2:
                        _emit_proj_chunk(ctx, u - 2, s)
                _emit_a2a(ctx, u)
            # Tail: projections of the last two units.
            for u in range(max(0, U - 2), U):
                _emit_yt_prefetch(ctx, u)
                for s in range(N_STRIPS):
                    _emit_proj_chunk(ctx, u, s)

    nc.compile()
    return nc


_NC_CACHE = {}


def _get_program(reps=1):
    if reps not in _NC_CACHE:
        _NC_CACHE[reps] = _build_program(reps)
    return _NC_CACHE[reps]


def _host_tables():
    inv_freq = 1.0 / (ROPE_BASE ** (np.arange(0, DH, 2, dtype=np.float32) / DH))
    t = np.arange(T, dtype=np.float32)
    freqs = np.outer(t, inv_freq).astype(np.float32)  # (T, 32)
    cos_t = np.cos(freqs).T                           # (32, T)
    sin_t = np.sin(freqs).T
    cos = np.empty((P, T), np.float32)
    sin = np.empty((P, T), np.float32)
    for blk in range(4):
        cos[blk * 32 : (blk + 1) * 32] = cos_t
        # rotate_half: row p<32 pairs with -q[p+32]; row p>=32 with +q[p-32]
        sgn = -1.0 if blk % 2 == 0 else 1.0
        sin[blk * 32 : (blk + 1) * 32] = sgn * sin_t
    return cos, sin


def make_in_maps(x, W_qkv, W_proj):
    x = np.asarray(x, np.float32).reshape(TOK, D)
    W_qkv = np.asarray(W_qkv, np.float32)
    W_proj = np.asarray(W_proj, np.float32)
    cos, sin = _host_tables()

    in_maps = []
    for c in range(N_CORES):
        j0 = c * P
        in_maps.append(
            {
                "x": x,
                "wq": np.ascontiguousarray(W_qkv[:, j0 : j0 + P]),
                "wk": np.ascontiguousarray(W_qkv[:, D + j0 : D + j0 + P]),
                "wv": np.ascontiguousarray(W_qkv[:, 2 * D + j0 : 2 * D + j0 + P]),
                "wp": W_proj,
                "cos": cos,
                "sin": sin,
                "ident": np.eye(P, dtype=np.float32),
            }
        )
    return in_maps


def kernel(x, W_qkv, W_proj):
    in_maps = make_in_maps(x, W_qkv, W_proj)
    nc = _get_program()
    res = run_bass_kernel_spmd(nc, in_maps, list(range(N_CORES)))
    return assemble([res.results[c]["out"] for c in range(N_CORES)])


def assemble(outs):
    full = np.empty((B, T, D), np.float32)
    for c in range(N_CORES):
        o = outs[c]
        for b in range(B):
            full[b, 256 * c : 256 * (c + 1)] = o[b * 256 : (b + 1) * 256]
    return full
